# revision 1
# baseline (speedup 1.0000x reference)
"""EntityPredictionHead_CandidateList Trainium2 kernel.

Math (full inputs):
    h = LayerNorm(gelu(hidden_states @ W_dense.T + b_dense)) * ln_gamma + ln_beta
    scores[b, c] = dot(decoder_table[cand_idx[b, c]], h[b]) + entity_bias[cand_idx[b, c]]

Sharding: batch (4096) split across 8 NeuronCores (512 rows each); dense
weights + full decoder table replicated per core.

Per-core device pipeline:
  1. Transform on PE/ACT/DVE -> h [512, 256], written to a DRAM bounce buffer.
  2. The 512*128 = 65536 candidate (b, v) pairs are sorted by v on the host
     and split into 128 blocks of 512 positions. Block k's vocab indices are
     expressed relative to a STATIC window base (SPACING*k - 8000, clipped), so
     they fit dma_gather's int16 index type; the program itself is
     data-independent.
  3. For each block: one dma_gather pulls the 512 table rows, a second pulls
     the 512 matching h rows (by batch index) from the bounce buffer. Both
     stripe position i -> (partition i%128, column i//128), so the pairing is
     positional. Gathers are spread over 4 SWDGE queues.
  4. DVE: elementwise product (in place) + segmented reduce -> 512 scores per
     block, accumulated in one [128, 512] tile, DMA'd out once.
  5. Host un-permutes scores back to (b, c) order and adds the bias gather.
"""

import numpy as np

import concourse.bacc as bacc
import concourse.mybir as mybir
import concourse.tile as tile
from concourse.bass_utils import run_bass_kernel_spmd
from concourse.masks import make_identity
from concourse.tile import add_dep_helper

# Problem shape (hardcoded per contract).
B = 4096
HIDDEN = 1024
EMB = 256
VOCAB = 500000
C = 128
EPS = 1e-12

N_CORES = 8
B_LOC = B // N_CORES           # 512 batch rows per core
P = 128
NBB = B_LOC // P               # 4 batch blocks per core
NPOS = B_LOC * C               # 65536 gather positions per core
BLK = 512                      # positions per gather instruction
NBLK = NPOS // BLK             # 128 blocks
COLS = BLK // P                # 4 columns per block
WIN = 32768                    # int16 index window
SPACING = VOCAB // NBLK        # 3906
BASE_MARGIN = 8000
N_QUEUES = 4

F32 = mybir.dt.float32
I16 = mybir.dt.int16


def block_base(k: int) -> int:
    return min(max(SPACING * k - BASE_MARGIN, 0), VOCAB - WIN)


def build_program():
    nc = bacc.Bacc(None, num_swdge_queues=N_QUEUES)

    hidden = nc.dram_tensor("hidden", [B_LOC, HIDDEN], F32, kind="ExternalInput")
    w = nc.dram_tensor("w", [EMB, HIDDEN], F32, kind="ExternalInput")
    bde = nc.dram_tensor("bde", [1, EMB], F32, kind="ExternalInput")
    gamma = nc.dram_tensor("gamma", [1, EMB], F32, kind="ExternalInput")
    beta = nc.dram_tensor("beta", [1, EMB], F32, kind="ExternalInput")
    table = nc.dram_tensor("table", [VOCAB, EMB], F32, kind="ExternalInput")
    # wrapped+replicated int16 index streams, 32 cols per block
    vidx = nc.dram_tensor("vidx", [P, NBLK * (BLK // 16)], I16,
                          kind="ExternalInput")
    bidx = nc.dram_tensor("bidx", [P, NBLK * (BLK // 16)], I16,
                          kind="ExternalInput")
    scores = nc.dram_tensor("scores", [P, NBLK * COLS], F32,
                            kind="ExternalOutput")

    with tile.TileContext(nc) as tc:
        with (
            tc.tile_pool(name="persist", bufs=1) as persist,
            tc.tile_pool(name="wload", bufs=1) as wload,
            tc.tile_pool(name="hid", bufs=2) as hidp,
            tc.tile_pool(name="ht", bufs=2) as htp,
            tc.tile_pool(name="small", bufs=1) as smallp,
            tc.tile_pool(name="gather", bufs=12) as gpool,
            tc.tile_pool(name="hgat", bufs=12) as hgpool,
            tc.tile_pool(name="acttrash", bufs=8) as trashp,
            tc.tile_pool(name="psum", bufs=2, space="PSUM") as psum,
            tc.tile_pool(name="psum_t", bufs=4, space="PSUM") as psum_t,
            tc.tile_pool(name="dram", bufs=1, space="DRAM") as dramp,
        ):
            # ---- prologue ----
            ident = persist.tile([P, P], F32)
            make_identity(nc, ident[:])

            ones_row = persist.tile([1, P], F32)
            nc.vector.memset(ones_row[:], 1.0)

            bde_sb = persist.tile([1, EMB], F32)
            nc.sync.dma_start(bde_sb[:], bde[:])
            gamma_sb = persist.tile([1, EMB], F32)
            nc.sync.dma_start(gamma_sb[:], gamma[:])
            beta_sb = persist.tile([1, EMB], F32)
            nc.sync.dma_start(beta_sb[:], beta[:])

            gamma_bc = persist.tile([P, EMB], F32)
            beta_bc = persist.tile([P, EMB], F32)
            for src, dst in ((gamma_sb, gamma_bc), (beta_sb, beta_bc)):
                pt = psum.tile([P, EMB], F32, space="PSUM", tag="bc")
                nc.tensor.matmul(pt[:], lhsT=ones_row[:], rhs=src[:],
                                 start=True, stop=True)
                nc.scalar.copy(dst[:], pt[:])

            eps_col = persist.tile([P, 1], F32)
            nc.vector.memset(eps_col[:], EPS)

            # W [256, 1024] -> wT chunks [k=128, e=256] x 8
            wT = persist.tile([P, HIDDEN // P * EMB], F32)
            for eb in range(EMB // P):
                wsb = wload.tile([P, HIDDEN], F32, tag="wsb")
                nc.sync.dma_start(wsb[:], w[eb * P:(eb + 1) * P, :])
                for kb in range(HIDDEN // P):
                    ptile = psum_t.tile([P, P], F32, space="PSUM", tag="tp")
                    nc.tensor.transpose(ptile[:], wsb[:, kb * P:(kb + 1) * P],
                                        ident[:])
                    nc.scalar.copy(
                        wT[:, kb * EMB + eb * P:kb * EMB + (eb + 1) * P],
                        ptile[:])

            # index streams
            vidx_sb = persist.tile([P, NBLK * (BLK // 16)], I16)
            nc.sync.dma_start(vidx_sb[:], vidx[:])
            bidx_sb = persist.tile([P, NBLK * (BLK // 16)], I16)
            nc.sync.dma_start(bidx_sb[:], bidx[:])

            # h bounce buffer in DRAM (gather source)
            h_dram = dramp.tile([B_LOC, EMB], F32)

            # ---- transform: h = LN(gelu(hidden @ W.T + b)) ----
            for bb in range(NBB):
                hid_sb = hidp.tile([P, HIDDEN], F32)
                nc.sync.dma_start(hid_sb[:], hidden[bb * P:(bb + 1) * P, :])

                hT = htp.tile([P, HIDDEN], F32)
                for kb in range(HIDDEN // P):
                    ptile = psum_t.tile([P, P], F32, space="PSUM", tag="tp")
                    nc.tensor.transpose(ptile[:], hid_sb[:, kb * P:(kb + 1) * P],
                                        ident[:])
                    nc.scalar.copy(hT[:, kb * P:(kb + 1) * P], ptile[:])

                ph = psum.tile([P, EMB], F32, space="PSUM", tag="ph")
                for kb in range(HIDDEN // P):
                    nc.tensor.matmul(
                        ph[:],
                        lhsT=hT[:, kb * P:(kb + 1) * P],
                        rhs=wT[:, kb * EMB:(kb + 1) * EMB],
                        start=(kb == 0), stop=False,
                    )
                nc.tensor.matmul(ph[:], lhsT=ones_row[:], rhs=bde_sb[:],
                                 start=False, stop=True)

                g_sb = htp.tile([P, EMB], F32, tag="g")
                sum_g = smallp.tile([P, 1], F32, tag="sumg")
                nc.scalar.activation(g_sb[:], ph[:],
                                     mybir.ActivationFunctionType.Gelu,
                                     accum_out=sum_g[:])
                mu = smallp.tile([P, 1], F32, tag="mu")
                nc.scalar.mul(mu[:], sum_g[:], 1.0 / EMB)

                cent = htp.tile([P, EMB], F32, tag="cent")
                nc.vector.tensor_scalar(cent[:], g_sb[:], mu[:], None,
                                        mybir.AluOpType.subtract)

                sq_trash = htp.tile([P, EMB], F32, tag="sqt")
                ssq = smallp.tile([P, 1], F32, tag="ssq")
                nc.scalar.activation(sq_trash[:], cent[:],
                                     mybir.ActivationFunctionType.Square,
                                     accum_out=ssq[:])
                std = smallp.tile([P, 1], F32, tag="std")
                nc.scalar.activation(std[:], ssq[:],
                                     mybir.ActivationFunctionType.Sqrt,
                                     bias=eps_col[:, 0:1], scale=1.0 / EMB)
                rstd = smallp.tile([P, 1], F32, tag="rstd")
                nc.vector.reciprocal(rstd[:], std[:])

                hfin = htp.tile([P, EMB], F32, tag="hfin")
                nc.vector.tensor_scalar(hfin[:], cent[:], rstd[:], None,
                                        mybir.AluOpType.mult)
                nc.vector.tensor_mul(hfin[:], hfin[:], gamma_bc[:])
                nc.vector.tensor_add(hfin[:], hfin[:], beta_bc[:])

                nc.sync.dma_start(h_dram[bb * P:(bb + 1) * P, :], hfin[:])

            # ---- gather + score blocks ----
            # The 8 SWDGE sem lanes rotate over Pool DMAs in final schedule
            # order; chain the gathers (order-only deps) so that order is
            # deterministic and queue := (n//2)%4 keeps each sem lane pinned
            # to a single SWDGE queue.
            sc_sb = persist.tile([P, NBLK * COLS], F32)
            icols = BLK // 16
            prev_gather = None

            def _chain(bass_inst):
                nonlocal prev_gather
                inst = bass_inst.ins
                if prev_gather is not None:
                    add_dep_helper(inst, prev_gather, sync=False,
                                   reason="pin SWDGE sem-lane/queue pairing")
                prev_gather = inst

            for k in range(NBLK):
                base = block_base(k)
                gt = gpool.tile([P, COLS * EMB], F32)
                _chain(nc.gpsimd.dma_gather(
                    gt[:].rearrange("p (c e) -> p c e", e=EMB),
                    table[base:base + WIN, :],
                    vidx_sb[:, k * icols:(k + 1) * icols],
                    BLK, BLK, EMB,
                    queue_num=(2 * k) % N_QUEUES,
                ))
                ht = hgpool.tile([P, COLS * EMB], F32)
                _chain(nc.gpsimd.dma_gather(
                    ht[:].rearrange("p (c e) -> p c e", e=EMB),
                    h_dram[:],
                    bidx_sb[:, k * icols:(k + 1) * icols],
                    BLK, BLK, EMB,
                    queue_num=(2 * k + 1) % N_QUEUES,
                ))
                gv = gt[:].rearrange("p (c e) -> p c e", e=EMB)
                nc.vector.tensor_tensor(gv, gv,
                                        ht[:].rearrange("p (c e) -> p c e",
                                                        e=EMB),
                                        op=mybir.AluOpType.mult)
                # segmented reduce offloaded to the (otherwise idle) Scalar
                # engine: Copy + accum_out sums the whole free dim per column
                for c in range(COLS):
                    trash = trashp.tile([P, EMB], F32)
                    nc.scalar.activation(
                        trash[:], gt[:, c * EMB:(c + 1) * EMB],
                        mybir.ActivationFunctionType.Copy,
                        accum_out=sc_sb[:, k * COLS + c:k * COLS + c + 1])

            nc.sync.dma_start(scores[:], sc_sb[:])

    nc.compile()
    return nc


_NC_CACHE = None


def _get_program():
    global _NC_CACHE
    if _NC_CACHE is None:
        _NC_CACHE = build_program()
    return _NC_CACHE


def _wrap_idx(vals):
    """[NPOS] int array -> [128, NBLK*32] int16 wrapped (i%16, i//16) per
    block and replicated across the 8 16-partition groups."""
    out = np.empty((P, NBLK * (BLK // 16)), dtype=np.int16)
    w = vals.reshape(NBLK, BLK // 16, 16).transpose(0, 2, 1)  # [NBLK, 16, 32]
    flat = w.transpose(1, 0, 2).reshape(16, NBLK * (BLK // 16))
    out[:] = np.tile(flat, (8, 1))
    return out


def make_in_maps(hidden_states, W_dense, b_dense, ln_gamma, ln_beta,
                 decoder_table, entity_bias, cand_idx):
    table = np.ascontiguousarray(decoder_table, dtype=np.float32)
    in_maps = []
    perms = []
    for cid in range(N_CORES):
        sl = slice(cid * B_LOC, (cid + 1) * B_LOC)
        cand = np.asarray(cand_idx[sl])
        v = cand.reshape(-1).astype(np.int64)            # position -> vocab id
        order = np.argsort(v, kind="stable")             # sorted positions
        v_sorted = v[order]
        b_sorted = (order // C).astype(np.int64)         # batch row per pos
        bases = np.array([block_base(k) for k in range(NBLK)], dtype=np.int64)
        local = v_sorted - np.repeat(bases, BLK)
        if local.min() < 0 or local.max() > WIN - 1:
            raise AssertionError(
                "candidate distribution does not fit static gather windows")
        in_maps.append({
            "hidden": np.ascontiguousarray(hidden_states[sl], dtype=np.float32),
            "w": np.asarray(W_dense, dtype=np.float32),
            "bde": np.asarray(b_dense, dtype=np.float32).reshape(1, EMB),
            "gamma": np.asarray(ln_gamma, dtype=np.float32).reshape(1, EMB),
            "beta": np.asarray(ln_beta, dtype=np.float32).reshape(1, EMB),
            "table": table,
            "vidx": _wrap_idx(local.astype(np.int16)),
            "bidx": _wrap_idx(b_sorted.astype(np.int16)),
        })
        perms.append(order)
    return in_maps, perms


def run(in_maps, trace=False):
    nc = _get_program()
    return run_bass_kernel_spmd(nc, in_maps, core_ids=list(range(N_CORES)),
                                trace=trace)


def unpermute_scores(raw, perm):
    """raw: device scores [128, NBLK*4]; score of sorted position i is at
    [i % 128, i // 128] within its block: i = k*512 + c*128 + p maps to
    [p, k*4 + c]. Return [B_LOC, C] in original order."""
    flat = raw.reshape(P, NBLK, COLS).transpose(1, 2, 0).reshape(-1)
    out = np.empty(NPOS, dtype=raw.dtype)
    out[perm] = flat
    return out.reshape(B_LOC, C)


def kernel(hidden_states, W_dense, b_dense, ln_gamma, ln_beta,
           decoder_table, entity_bias, cand_idx):
    in_maps, perms = make_in_maps(hidden_states, W_dense, b_dense, ln_gamma,
                                  ln_beta, decoder_table, entity_bias,
                                  cand_idx)
    res = run(in_maps)
    bias = np.asarray(entity_bias, dtype=np.float32)[np.asarray(cand_idx)]
    parts = [unpermute_scores(res.results[i]["scores"], perms[i])
             for i in range(N_CORES)]
    return np.concatenate(parts, axis=0) + bias

